# revision 2
# baseline (speedup 1.0000x reference)
"""GCN message-passing layer on 8 Trainium2 NeuronCores (Bass/Tile).

Strategy
--------
Edges are bucketed by destination node. Node rows are split across the 8
cores (6250 destination nodes per core), so each core owns the full
reduction for its nodes and no cross-core collective is needed. Within a
core, destination nodes are processed in chunks of 128; the segment-sum
over each chunk's edges runs on the tensor engine as a sequence of
one-hot matmuls accumulating in PSUM:

    aggT[f, n] += sum_e msgs[e, f] * onehot[e, n]
    onehot[e, n] = (weight[e] + 1) * (dst_rel[e] == n)

Messages are gathered per-edge from a replicated bf16 feature table with
`dma_gather` (int16 indices, so the table is split in two halves of 25000
rows each), spread over all 4 SWDGE queues. The SWDGE descriptor ring
(dynamic_dma_scratch_size) is sized so every descriptor fits in the ring
at once: descriptor generation on GpSimd never stalls on DMA drain and
runs ahead of the consuming matmuls.

The scaled one-hot tiles are generated ON-CHIP on the vector engine (two
tensor_tensor passes over broadcast access patterns:
  oh = is_equal(iota_cols, dst_rel)  then  oh *= (w+1)
) instead of streaming ~29MB of precomputed tiles from HBM — the DMA
engines' packet throughput is the kernel's critical resource. Padded
edge slots carry wp1 == 0 so they vanish; pad gather indices are 0 (a
valid, ignored row). The epilogue per chunk adds the bf16 self-term
(precomputed on host as (feature * (self_weight+1)).T), applies the
final linear in bf16 on the tensor engine, and adds the bias via the
scalar engine's per-partition bias during the PSUM->SBUF copy. Output is
written transposed ([128, 6272] per core) and rearranged on host.
"""

import sys

for _p in ("/opt/trn_rl_repo",):
    if _p not in sys.path:
        sys.path.insert(0, _p)

import ml_dtypes
import numpy as np

N = 50000
E = 800000
F = 128
NCORES = 8
P = 128
CW = 128                      # destination-chunk width (PSUM free dim)
NLOC = N // NCORES            # 6250 destination nodes per core
NCHUNK = (NLOC + CW - 1) // CW
NPAD = NCHUNK * CW
SPLIT = N // 2                # feature-table split so gather indices fit int16
GC = 2                        # chunks batched per dma_gather
NSWQ = 4                      # SWDGE queues (Q7 core pairs) for parallel desc-gen

# gather groups: (start_chunk, n_chunks); last group takes the remainder
GROUPS = [(s, min(GC, NCHUNK - s)) for s in range(0, NCHUNK, GC)]

_cache: dict = {}


def _host_pack(inputs):
    feature = np.asarray(inputs["feature"], np.float32)
    sw = np.asarray(inputs["self_weight"], np.float32)
    w = np.asarray(inputs["weight"], np.float32)
    src = np.asarray(inputs["src"]).astype(np.int64)
    dst = np.asarray(inputs["dst"]).astype(np.int64)
    W = np.asarray(inputs["W"], np.float32)
    b = np.asarray(inputs["b"], np.float32)

    core = dst // NLOC
    dst_loc = dst - core * NLOC
    chunk = dst_loc // CW
    dst_rel = dst_loc - chunk * CW
    half = (src >= SPLIT).astype(np.int64)
    src_rel = (src - half * SPLIT).astype(np.int16)
    wp1v = (w + 1.0).astype(np.float32)

    gid = (core * NCHUNK + chunk) * 2 + half
    order = np.argsort(gid, kind="stable")
    counts = np.bincount(gid, minlength=NCORES * NCHUNK * 2)
    T = max(1, int(np.ceil(counts.max() / P)))
    S = T * P
    M = NCHUNK * 2 * T  # one-hot tiles (= matmuls) per core

    starts = np.zeros(NCORES * NCHUNK * 2 + 1, np.int64)
    np.cumsum(counts, out=starts[1:])
    gs = gid[order]
    pos = np.arange(E, dtype=np.int64) - starts[gs]
    ci = gs // (NCHUNK * 2)
    rem = gs % (NCHUNK * 2)
    ch = rem // 2
    hf = rem % 2

    idx_a = np.zeros((NCORES, NCHUNK, 2, S), np.int16)
    wp1_a = np.zeros((NCORES, NCHUNK, 2, S), np.float32)
    drel_a = np.zeros((NCORES, NCHUNK, 2, S), np.int64)
    idx_a[ci, ch, hf, pos] = src_rel[order]
    wp1_a[ci, ch, hf, pos] = wp1v[order]
    drel_a[ci, ch, hf, pos] = dst_rel[order]

    bf = ml_dtypes.bfloat16
    flo_np = np.ascontiguousarray(feature[:SPLIT].astype(bf))
    fhi_np = np.ascontiguousarray(feature[SPLIT:].astype(bf))
    wt_np = np.ascontiguousarray(W.T.astype(bf))
    b_np = np.ascontiguousarray(b.reshape(P, 1).astype(np.float32))
    iota_np = np.ascontiguousarray(
        np.broadcast_to(np.arange(CW, dtype=np.float32), (P, CW)).astype(bf)
    )

    def wrap_idx(half_idx):
        # half_idx: [NCHUNK, S] int16, chunk-major edge slots for one table half.
        # dma_gather consumes indices wrapped in 16 partitions (replicated x8):
        # within each gather group, element i lives at [i % 16, i // 16].
        blocks = []
        for s0, gn in GROUPS:
            flat = half_idx[s0 : s0 + gn].reshape(gn * S)
            wr = np.tile(flat.reshape((gn * S) // 16, 16).T, (8, 1))
            blocks.append(wr)
        return np.ascontiguousarray(np.concatenate(blocks, axis=1))

    in_maps = []
    for c in range(NCORES):
        fs = feature[c * NLOC : (c + 1) * NLOC] * (sw[c * NLOC : (c + 1) * NLOC] + 1.0)
        feats_np = np.zeros((P, NPAD), bf)
        feats_np[:, :NLOC] = fs.T.astype(bf)
        # per-tile slot metadata for on-chip one-hot generation: column m is
        # tile (ch*2 + hf)*T + t, partition p is the slot within the tile.
        # Padded slots have wp1 == 0 -> zero one-hot row.
        drel_np = np.ascontiguousarray(
            drel_a[c].reshape(M, P).T.astype(np.float32).astype(bf)
        )
        wp1_np = np.ascontiguousarray(
            wp1_a[c].reshape(M, P).T.astype(bf)
        )
        in_maps.append(
            {
                "flo": flo_np,
                "fhi": fhi_np,
                "feats": feats_np,
                "idxlo": wrap_idx(idx_a[c, :, 0, :]),
                "idxhi": wrap_idx(idx_a[c, :, 1, :]),
                "iota": iota_np,
                "drel": drel_np,
                "wp1m": wp1_np,
                "wt": wt_np,
                "bvec": b_np,
            }
        )
    return T, in_maps


_patched_sem_assign = False


def _patch_sem_assignment():
    """Partition Tile's 8 DMASW sem lanes by SWDGE queue (2 lanes per queue).

    The hardware locks each DMASW semaphore to one SWDGE queue (shadow-sem
    tracking), but Tile's round-robin lane assignment is queue-unaware, so
    multi-queue dma_gather programs get sems shared across queues. Keyed off
    each Pool-DMA instruction's queue_num field instead.
    """
    global _patched_sem_assign
    if _patched_sem_assign:
        return
    import concourse.tile_sem_assignment as tsa
    from concourse import bass_isa, mybir

    orig = tsa.TileClockTick._assign_tick

    def _assign_tick_qaware(self, inst):
        qn = getattr(inst, "queue_num", None)
        if (
            qn is not None
            and isinstance(inst, tsa.DMAInst)
            and not isinstance(inst, bass_isa.UserSyncedRemoteDMADescs)
            and inst.engine == mybir.EngineType.Pool
        ):
            rr = getattr(self, "_q_rr", None)
            if rr is None:
                rr = self._q_rr = {}
            lane = 2 * qn + rr.get(qn, 0)
            rr[qn] = 1 - rr.get(qn, 0)
            self.next_sw_dma_idx = lane
        return orig(self, inst)

    tsa.TileClockTick._assign_tick = _assign_tick_qaware
    _patched_sem_assign = True


def _build(T):
    import concourse.bacc as bacc
    import concourse.mybir as mybir
    import concourse.tile as tile

    _patch_sem_assignment()

    fp32 = mybir.dt.float32
    bf16 = mybir.dt.bfloat16
    i16 = mybir.dt.int16
    M = NCHUNK * 2 * T

    nc = bacc.Bacc(
        "TRN2",
        target_bir_lowering=False,
        debug=False,
        num_swdge_queues=NSWQ,
        # Ring sized so every gather descriptor fits at once (per queue per
        # engine per direction: ceil(50/4) calls x (2304/16 + 1) descs x 16B
        # = ~30KB); desc-gen never stalls waiting for DMA drain.
        dynamic_dma_scratch_size=32768,
    )
    flo = nc.dram_tensor("flo", [SPLIT, F], bf16, kind="ExternalInput").ap()
    fhi = nc.dram_tensor("fhi", [N - SPLIT, F], bf16, kind="ExternalInput").ap()
    feats = nc.dram_tensor("feats", [P, NPAD], bf16, kind="ExternalInput").ap()
    idxlo = nc.dram_tensor("idxlo", [P, NCHUNK * T * 8], i16, kind="ExternalInput").ap()
    idxhi = nc.dram_tensor("idxhi", [P, NCHUNK * T * 8], i16, kind="ExternalInput").ap()
    iota = nc.dram_tensor("iota", [P, CW], bf16, kind="ExternalInput").ap()
    drel = nc.dram_tensor("drel", [P, M], bf16, kind="ExternalInput").ap()
    wp1m = nc.dram_tensor("wp1m", [P, M], bf16, kind="ExternalInput").ap()
    wt = nc.dram_tensor("wt", [F, F], bf16, kind="ExternalInput").ap()
    bvec = nc.dram_tensor("bvec", [P, 1], fp32, kind="ExternalInput").ap()
    outT = nc.dram_tensor("outT", [P, NPAD], fp32, kind="ExternalOutput").ap()

    with tile.TileContext(nc) as tc:
        with (
            tc.tile_pool(name="const", bufs=1) as cp,
            tc.tile_pool(name="msgs", bufs=8) as mp,
            tc.tile_pool(name="oh", bufs=4) as ohp,
            tc.tile_pool(name="ep", bufs=3) as ep,
            tc.tile_pool(name="psA", bufs=2, space="PSUM") as psA,
            tc.tile_pool(name="psB", bufs=2, space="PSUM") as psB,
        ):
            feats_sb = cp.tile([P, NPAD], bf16)
            nc.sync.dma_start(out=feats_sb[:], in_=feats[:, :])
            idxlo_sb = cp.tile([P, NCHUNK * T * 8], i16)
            nc.sync.dma_start(out=idxlo_sb[:], in_=idxlo[:, :])
            idxhi_sb = cp.tile([P, NCHUNK * T * 8], i16)
            nc.sync.dma_start(out=idxhi_sb[:], in_=idxhi[:, :])
            iota_sb = cp.tile([P, CW], bf16)
            nc.sync.dma_start(out=iota_sb[:], in_=iota[:, :])
            drel_sb = cp.tile([P, M], bf16)
            nc.sync.dma_start(out=drel_sb[:], in_=drel[:, :])
            wp1_sb = cp.tile([P, M], bf16)
            nc.sync.dma_start(out=wp1_sb[:], in_=wp1m[:, :])
            wt_sb = cp.tile([F, F], bf16)
            nc.sync.dma_start(out=wt_sb[:], in_=wt[:, :])
            b_sb = cp.tile([P, 1], fp32)
            nc.sync.dma_start(out=b_sb[:], in_=bvec[:, :])

            qrr = 0      # round-robin SWDGE queue assignment
            idx_col = 0  # running int16 idx column offset (shared by lo/hi)
            for s0, gn in GROUPS:
                w = gn * T * 8
                ni = gn * T * P
                m0 = s0 * 2 * T
                mg = gn * 2 * T
                mlo = mp.tile([P, gn * T, F], bf16, tag="mlo")
                nc.gpsimd.dma_gather(
                    mlo[:, :, :], flo[:, :],
                    idxlo_sb[:, idx_col : idx_col + w],
                    ni, ni, F,
                    single_packet=False,  # >64 descriptors per SDMA lane
                    queue_num=qrr % NSWQ,
                )
                qrr += 1
                mhi = mp.tile([P, gn * T, F], bf16, tag="mhi")
                nc.gpsimd.dma_gather(
                    mhi[:, :, :], fhi[:, :],
                    idxhi_sb[:, idx_col : idx_col + w],
                    ni, ni, F,
                    single_packet=False,
                    queue_num=qrr % NSWQ,
                )
                qrr += 1
                idx_col += w
                # on-chip scaled one-hot: oh[p, m, j] = (j == drel[p, m]) * wp1[p, m]
                ohg = ohp.tile([P, mg, CW], bf16, tag="ohg")
                iota_b = iota_sb[:].unsqueeze(1).broadcast_to([P, mg, CW])
                drel_b = (
                    drel_sb[:, m0 : m0 + mg].unsqueeze(2).broadcast_to([P, mg, CW])
                )
                wp1_b = (
                    wp1_sb[:, m0 : m0 + mg].unsqueeze(2).broadcast_to([P, mg, CW])
                )
                nc.vector.tensor_tensor(
                    out=ohg[:, :, :], in0=iota_b, in1=drel_b,
                    op=mybir.AluOpType.is_equal,
                )
                nc.vector.tensor_tensor(
                    out=ohg[:, :, :], in0=ohg[:, :, :], in1=wp1_b,
                    op=mybir.AluOpType.mult,
                )
                for cc in range(gn):
                    c = s0 + cc
                    agg = psA.tile([P, CW], fp32)
                    n_mm = 2 * T
                    k = 0
                    for hf, msrc in ((0, mlo), (1, mhi)):
                        for t in range(T):
                            mloc = (cc * 2 + hf) * T + t
                            nc.tensor.matmul(
                                out=agg[:],
                                lhsT=msrc[:, cc * T + t, :],
                                rhs=ohg[:, mloc, :],
                                start=(k == 0),
                                stop=(k == n_mm - 1),
                            )
                            k += 1
                    hT = ep.tile([P, CW], bf16, tag="hT")
                    nc.vector.tensor_tensor(
                        out=hT[:], in0=agg[:],
                        in1=feats_sb[:, c * CW : (c + 1) * CW],
                        op=mybir.AluOpType.add,
                    )
                    ops = psB.tile([P, CW], fp32)
                    nc.tensor.matmul(out=ops[:], lhsT=wt_sb[:], rhs=hT[:], start=True, stop=True)
                    oc = ep.tile([P, CW], fp32, tag="oc")
                    nc.scalar.activation(
                        out=oc[:], in_=ops[:],
                        func=mybir.ActivationFunctionType.Identity,
                        bias=b_sb[:, 0:1], scale=1.0,
                    )
                    nc.sync.dma_start(out=outT[:, c * CW : (c + 1) * CW], in_=oc[:])
    nc.compile()
    return nc


def _get_program(T):
    if T not in _cache:
        _cache[T] = _build(T)
    return _cache[T]


def kernel(**inputs) -> np.ndarray:
    import concourse.bass_utils as bass_utils

    T, in_maps = _host_pack(inputs)
    nc = _get_program(T)
    # Warmup execution: the very first NEFF execution after device bringup
    # has produced corrupted gather results; run twice and keep the second.
    bass_utils.run_bass_kernel_spmd(nc, in_maps, core_ids=list(range(NCORES)))
    res = bass_utils.run_bass_kernel_spmd(nc, in_maps, core_ids=list(range(NCORES)))
    out = np.empty((N, F), np.float32)
    for c in range(NCORES):
        out[c * NLOC : (c + 1) * NLOC] = res.results[c]["outT"][:, :NLOC].T
    return out
